# revision 1
# baseline (speedup 1.0000x reference)
"""Bass/Trainium2 kernel for batched multi-head self-attention.

Module math (per batch b):
    q = vec @ Wq; k = vec @ Wk; v = vec @ Wv            (per head h, dim d=16)
    S = q k^T / sqrt(d);  P = softmax_j(S);  recv = P v
    out = recv @ Wo

Sharding: data-parallel over batch (8 batches -> 8 NeuronCores), weights
replicated. Each core runs an identical Bass program on its vec slice.

Per-head pipeline on a core:
  1. form1: S[i, j] via K=64 strip fp16 matmuls; DVE row-max (negated).
  2. "m-dance": the per-row -max vector is transposed (PE) and DMA-flattened
     into an augmentation partition of the fp16 Q^T tensor.
  3. S'^T[j, i] = KT-aug.T @ QT-aug (K=64 strip fp16): the aug row
     (ones x -max) subtracts the row max inside the matmul, so ACT exp with
     scale=1/4 needs no per-column bias. exp -> P^T in fp16.
  4. PV: lhsT = [V_h | 1] fp16 (M=17) accumulates recv^T plus the softmax
     denominator in one stream, col-tiled across 3 PSUM strips.
  5. Tail: 128-partition reciprocal + PE expand-matmul + fused normalize,
     Wo projection.

Strip layout (64 rows/head, used by both S passes; matmul cost is
free-dim-bound so the extra K rows are free):
  0-15  q/k hi (fp16 RNE round of the fp32 projection)
  16    aug row (ones for KT, -rowmax for QT)
  17-30 +/- replicated head dims: contributions cancel exactly but keep
        the PE HAM activity monitor fed (else it clock-gates to 1.2 GHz)
  32-47 QT: q residual fp16(q - hi)  | KT: k hi copy
  48-63 QT: q hi copy                | KT: k residual
The three live row-bands give S = qhi*khi + qlo*khi + qhi*klo: ~22-bit
precision at fp16 matmul rate (1 cycle/row vs 4 for fp32).

Shapes (hardcoded): vec [8, 1024, 128]; Wq/Wk/Wv [128, 8, 16]; Wo [8, 16, 128].
"""

import sys

sys.path.insert(0, "/opt/trn_rl_repo")

from contextlib import ExitStack

import numpy as np

import concourse.bacc as bacc
import concourse.tile as tile
from concourse import mybir
from concourse.bass_utils import run_bass_kernel_spmd
from concourse.masks import make_identity

F32 = mybir.dt.float32
F32R = mybir.dt.float32r
F16 = mybir.dt.float16
BF16 = mybir.dt.bfloat16
Exp = mybir.ActivationFunctionType.Exp

B, N, X, H, D = 8, 1024, 128, 8, 16
NCHUNK = N // 128          # 8 chunks of 128 along the token dim
SCALE = 0.25               # 1/sqrt(16)
NR = 4                     # qk rounds: 2 heads each at strips {0, 64}

_CACHED_NC = None


def build_nc():
    """Build the per-core Bass program (identical on all cores)."""
    nc = bacc.Bacc("TRN2")

    # DRAM I/O. Weight tensors arrive pre-permuted from numpy (see kernel()).
    d_wq = [nc.dram_tensor(f"wq{r}", (X, 128), F32, kind="ExternalInput")
            for r in range(NR)]
    d_wk = [nc.dram_tensor(f"wk{r}", (X, 128), F32, kind="ExternalInput")
            for r in range(NR)]
    d_wv = nc.dram_tensor("wv", (X, 128), F32, kind="ExternalInput")
    d_wo = nc.dram_tensor("wo", (128, X), F32, kind="ExternalInput")
    d_vec = nc.dram_tensor("vec", (N, X), F32, kind="ExternalInput")
    d_e8 = nc.dram_tensor("e8c", (H, 128), F32, kind="ExternalInput")
    d_ones = nc.dram_tensor("ones", (1, N), F16, kind="ExternalInput")
    d_out = nc.dram_tensor("out", (N, X), F32, kind="ExternalOutput")

    with tile.TileContext(nc) as tc, ExitStack() as top:
        const = top.enter_context(tc.tile_pool(name="const", bufs=1))
        ident = const.tile([128, 128], F32)
        make_identity(nc, ident)

        w_sb = {}
        for name, dram in ([(f"wq{r}", d_wq[r]) for r in range(NR)]
                           + [(f"wk{r}", d_wk[r]) for r in range(NR)]
                           + [("wv", d_wv), ("wo", d_wo)]):
            t = const.tile([128, 128], F32, tag=f"w_{name}", name=f"w_{name}")
            eng = [nc.sync, nc.scalar, nc.gpsimd][len(w_sb) % 3]
            eng.dma_start(out=t[:], in_=dram[:, :])
            w_sb[name] = t

        vecT = const.tile([128, N], F32, tag="vecT")      # [x, n]
        # QT/KT layout per round r (heads 2r, 2r+1): strip t=h%2 occupies
        # partitions [64t, 64t+17): rows 64t+d hold head dim d, row 64t+16
        # is the aug row (ones for KT, -rowmax for QT); rows 64t+17..64t+63
        # are zeros (K=64 padding).
        # fp16 Q^T/K^T: single RNE rounding off the fp32 projection. Strip
        # pad rows carry +/- replicated q/k pairs (packed into the weights)
        # whose matmul contributions cancel exactly; they exist to keep the
        # PE HAM activity monitor from clock-gating to 1.2 GHz.
        QT = {r: const.tile([128, N], F16, tag=f"qt{r}", name=f"qt{r}")
              for r in range(NR)}
        KT = {r: const.tile([128, N], F16, tag=f"kt{r}", name=f"kt{r}")
              for r in range(NR)}
        # V layout: [128 j-in-chunk, jc, 17*h + d], col 17h+16 = ones.
        V_sb = const.tile([128, NCHUNK, 17 * H], F16, tag="vsb")
        pt_pool = top.enter_context(tc.tile_pool(name="pt", bufs=3))
        # raw recv output (incl. den rows): head h -> raw[h//3],
        # psum col strip 32*(h%3).
        raw = {r: const.tile([128, N], F32, tag=f"raw{r}", name=f"raw{r}")
               for r in range(3)}
        recvT = const.tile([128, N], F32, tag="recvT")     # [(h d), i]
        recvN = const.tile([128, N], F32, tag="recvN")     # normalized
        # denominators packed over all 128 partitions: head h, token i ->
        # den128[16h + i//64, i%64]; reciprocal there is 16x cheaper than
        # on an [8, N] layout, then DMA'd back to [8, N] for the expand.
        den128 = const.tile([128, 64], F32, tag="den128")
        rden128 = const.tile([128, 64], F32, tag="rden128")
        rden = const.tile([H, N], F32, tag="rden")
        e8 = const.tile([H, 128], F32, tag="e8")           # expand matrix
        mha_sb = const.tile([128, NCHUNK, X], F32, tag="mha")

        nc.sync.dma_start(out=e8[:], in_=d_e8[:, :])
        v_heads = V_sb[:].rearrange("p c (h s) -> p c h s", h=H)
        nc.vector.memset(v_heads[:, :, :, 16:17], 1.0)

        # ---- Phase 0: vecT via PE transposes; projections. ----
        with tc.tile_pool(name="stage", bufs=3) as stage, \
                tc.tile_pool(name="ps0", bufs=2, space="PSUM") as ps0, \
                tc.tile_pool(name="ps0b", bufs=2, space="PSUM") as ps0b:
            for c in range(NCHUNK):
                vt = stage.tile([128, 128], F32, tag="vstage")
                nc.sync.dma_start(out=vt[:], in_=d_vec[c * 128:(c + 1) * 128, :])
                pt_ = ps0b.tile([128, 128], F32, tag="trp")
                nc.tensor.transpose(pt_[:, :], vt[:], ident[:])
                nc.scalar.copy(vecT[:, c * 128:(c + 1) * 128], pt_[:, :])

            # QT/KT projections: psum = W.T @ vecT  -> [hd-pos, n]
            # fp16 hi/lo split: the plain copy rounds every strip row to
            # fp16 (hi); a tensor_tensor subtract then overwrites the
            # residual rows (q side: 17-32, k side: 33-48) with
            # fp16(p - hi), giving ~22-bit S at fp16 matmul speed.
            for rnd in range(NR):
                for wname, dst, is_q in ((f"wq{rnd}", QT[rnd], True),
                                         (f"wk{rnd}", KT[rnd], False)):
                    p = ps0.tile([128, N], F32, tag="proj")
                    for half in range(2):
                        sl = slice(half * 512, (half + 1) * 512)
                        nc.tensor.matmul(p[:, sl], w_sb[wname][:],
                                         vecT[:, sl], start=True, stop=True)
                    nc.scalar.copy(dst[:, :], p[:, :])
                    for t in range(2):
                        b = 64 * t
                        if is_q:
                            # residual rows 32-47 (16-row TT, 32-aligned)
                            rs = slice(b + 32, b + 48)
                            nc.vector.tensor_tensor(
                                dst[rs, :], p[rs, :], dst[rs, :],
                                op=mybir.AluOpType.subtract)
                        else:
                            # residual wanted at 48-63; only 32-aligned
                            # windows are legal, so TT 32-63 then restore
                            # hi over 32-47.
                            rs = slice(b + 32, b + 64)
                            nc.vector.tensor_tensor(
                                dst[rs, :], p[rs, :], dst[rs, :],
                                op=mybir.AluOpType.subtract)
                            nc.scalar.copy(dst[b + 32:b + 48, :],
                                           p[b + 32:b + 48, :])
            # ones rows of KT aug partitions
            for rnd in range(NR):
                for t in range(2):
                    nc.sync.dma_start(
                        out=KT[rnd][64 * t + 16:64 * t + 17, :],
                        in_=d_ones[:, :])

            # V projection: per chunk [j, hd] = vecT[:,chunk].T @ Wv
            for c in range(NCHUNK):
                pv = ps0b.tile([128, 128], F32, tag="projv")
                nc.tensor.matmul(pv[:, :], vecT[:, c * 128:(c + 1) * 128],
                                 w_sb["wv"][:], start=True, stop=True)
                dst = V_sb[:, c, :].rearrange("p (h s) -> p h s", h=H)
                src = pv[:, :].rearrange("p (h d) -> p h d", h=H)
                nc.vector.tensor_copy(dst[:, :, 0:16], src[:])

        # ---- Main loop over heads. ----
        with tc.tile_pool(name="small", bufs=6) as small, \
                tc.tile_pool(name="psm", bufs=3, space="PSUM") as psm, \
                tc.tile_pool(name="psr", bufs=2, space="PSUM") as psr:
            def emit_form1(rnd, c, m_hs):
                """One i-chunk of the f32r max-pass for both heads of rnd."""
                qt_, kt_ = QT[rnd], KT[rnd]
                f1s = {}
                for h in (2 * rnd, 2 * rnd + 1):
                    sp = 64 * (h % 2)
                    f1 = psm.tile([128, N], F32, tag="big",
                                  name=f"f1_{h}_{c}")
                    f1s[h] = f1
                    for half in range(2):
                        sl = slice(half * 512, (half + 1) * 512)
                        nc.tensor.matmul(
                            f1[:, sl],
                            qt_[sp:sp + 64, c * 128:(c + 1) * 128],
                            kt_[sp:sp + 64, sl], start=True, stop=True)
                for h in (2 * rnd, 2 * rnd + 1):
                    nc.vector.tensor_reduce(
                        m_hs[h][:, c:c + 1], f1s[h][:, :],
                        axis=mybir.AxisListType.X,
                        op=mybir.AluOpType.max, negate=True)

            def new_mhs(rnd):
                return {h: small.tile([128, NCHUNK], F32, tag="mh",
                                      name=f"mh{h}")
                        for h in (2 * rnd, 2 * rnd + 1)}

            # prologue: round 0 max-pass
            m_cur = new_mhs(0)
            for c in range(NCHUNK):
                emit_form1(0, c, m_cur)

            for rnd in range(NR):
                pair = (2 * rnd, 2 * rnd + 1)
                qt, kt = QT[rnd], KT[rnd]

                # m-dance per head: -rowmax -> aug row of QT.
                for h in pair:
                    sp = 64 * (h % 2)
                    trp = psr.tile([128, 512], F32, tag="recv",
                                   name=f"trp{h}")
                    nc.tensor.transpose(trp[0:NCHUNK, 0:128],
                                        m_cur[h][:], ident[:])
                    m8 = small.tile([NCHUNK, 128], F16, tag="m8",
                                    name=f"m8_{h}")
                    nc.scalar.copy(m8[:], trp[0:NCHUNK, 0:128])
                    nc.sync.dma_start(out=qt[sp + 16:sp + 17, :], in_=m8[:])

                # S'^T + exp, strip-interleaved across the head pair,
                # with next round's max-pass chunks woven in.
                m_nxt = new_mhs(rnd + 1) if rnd + 1 < NR else None
                PTs = {h: pt_pool.tile([128, NCHUNK * N], F16, tag="pt",
                                       name=f"pt{h}")
                       for h in pair}
                for jc in range(NCHUNK):
                    sts = {}
                    for h in pair:
                        sp = 64 * (h % 2)
                        st = psm.tile([128, N], F32, tag="big",
                                      name=f"st_{h}_{jc}")
                        sts[h] = st
                        for half in range(2):
                            sl = slice(half * 512, (half + 1) * 512)
                            nc.tensor.matmul(
                                st[:, sl],
                                kt[sp:sp + 64, jc * 128:(jc + 1) * 128],
                                qt[sp:sp + 64, sl], start=True, stop=True)
                    for h in pair:
                        nc.scalar.activation(
                            PTs[h][:, jc * N:jc * N + N], sts[h][:, :],
                            Exp, bias=0.0, scale=SCALE)
                    if m_nxt is not None:
                        emit_form1(rnd + 1, jc, m_nxt)

                # PV for both heads (different PSUM col strips); extract
                # recv rows + denominators as soon as each half lands.
                for half in range(2):
                    prvs = {}
                    for h in pair:
                        cs = 32 * (h % 3)
                        prv = psr.tile([128, 512], F32, tag="recv",
                                       name=f"prv{h}_{half}")
                        prvs[h] = prv
                        for jc in range(NCHUNK):
                            nc.tensor.matmul(
                                prv[cs:cs + 17, :],
                                V_sb[:, jc, 17 * h:17 * h + 17],
                                PTs[h][:, jc * N + half * 512:
                                        jc * N + (half + 1) * 512],
                                start=(jc == 0), stop=(jc == NCHUNK - 1))
                    for h in pair:
                        cs = 32 * (h % 3)
                        rv = raw[h // 3]
                        hs = slice(half * 512, (half + 1) * 512)
                        nc.scalar.copy(rv[cs:cs + 17, hs],
                                       prvs[h][cs:cs + 17, :])
                        nc.sync.dma_start(out=recvT[16 * h:16 * h + 16, hs],
                                          in_=rv[cs:cs + 16, hs])
                        dp = 16 * h + 8 * half
                        nc.sync.dma_start(out=den128[dp:dp + 8, :],
                                          in_=rv[cs + 16:cs + 17, hs])
                m_cur = m_nxt

        # ---- Tail: normalize + output projection. ----
        with tc.tile_pool(name="pst", bufs=2, space="PSUM") as pst, \
                tc.tile_pool(name="pstb", bufs=2, space="PSUM") as pstb:
            nc.vector.reciprocal(rden128[:], den128[:])
            for h in range(H):
                nc.sync.dma_start(out=rden[h:h + 1, :],
                                  in_=rden128[16 * h:16 * h + 16, :])
            pe_ = pst.tile([128, N], F32, tag="expand")
            for half in range(2):
                sl = slice(half * 512, (half + 1) * 512)
                nc.tensor.matmul(pe_[:, sl], e8[:], rden[:, sl],
                                 start=True, stop=True)
            nc.vector.tensor_mul(recvN[:], recvT[:], pe_[:, :])
            for c in range(NCHUNK):
                po = pstb.tile([128, 128], F32, tag="mha")
                nc.tensor.matmul(po[:, :], recvN[:, c * 128:(c + 1) * 128],
                                 w_sb["wo"][:], start=True, stop=True)
                nc.scalar.copy(mha_sb[:, c, :], po[:, :])
                nc.sync.dma_start(out=d_out[c * 128:(c + 1) * 128, :],
                                  in_=mha_sb[:, c, :])

    nc.finalize()
    return nc


def _permute_weights(Wq, Wk, Wv, Wo):
    """Numpy-side weight layout prep: strip-pack with +/- replica padding.

    Strip cols 17+s / 40+s (s<23) carry replicated head dims (q: same sign
    both; k: opposite signs) so their S contributions cancel exactly while
    keeping the PE array's activity monitor fed (avoids 1.2 GHz clock-gate).
    Col 16 stays zero (aug slot), col 63 zero.
    """
    def strip_pack(W, heads, neg_second):
        out = np.zeros((X, 128), dtype=np.float32)
        for t, h in enumerate(heads):
            base = 64 * t
            # three copies of the head dims: hi term + residual source +
            # hi-again (rows 17-32 / 33-48 become hi/lo split on device)
            out[:, base:base + 16] = W[:, h, :]
            out[:, base + 32:base + 48] = W[:, h, :]
            out[:, base + 48:base + 64] = W[:, h, :]
            # HAM-activity pads at 17-30: 7 cancelling pairs
            for s in range(7):
                out[:, base + 17 + s] = W[:, h, s]
                out[:, base + 24 + s] = (-1.0 if neg_second else 1.0) \
                    * W[:, h, s]
        return out

    e8c = np.zeros((H, 128), dtype=np.float32)
    for h in range(H):
        e8c[h, 16 * h:16 * h + 16] = 1.0
    d = dict(
        wv=np.ascontiguousarray(Wv.reshape(X, 128)),
        wo=np.ascontiguousarray(Wo.reshape(128, X)),
        e8c=e8c, ones=np.ones((1, N), dtype=np.float16),
    )
    for r in range(NR):
        d[f"wq{r}"] = strip_pack(Wq, [2 * r, 2 * r + 1], False)
        d[f"wk{r}"] = strip_pack(Wk, [2 * r, 2 * r + 1], True)
    return d


def kernel(Wq, Wk, Wv, Wo, vec, trace=False):
    global _CACHED_NC
    if _CACHED_NC is None:
        _CACHED_NC = build_nc()
    nc = _CACHED_NC

    w = _permute_weights(np.asarray(Wq, np.float32), np.asarray(Wk, np.float32),
                         np.asarray(Wv, np.float32), np.asarray(Wo, np.float32))
    vec = np.asarray(vec, np.float32)
    in_maps = [dict(w, vec=np.ascontiguousarray(vec[b])) for b in range(B)]
    res = run_bass_kernel_spmd(nc, in_maps, core_ids=list(range(B)),
                               trace=trace)
    out = np.stack([res.results[b]["out"] for b in range(B)])
    if trace:
        return out, res
    return out



# revision 5
# speedup vs baseline: 1.1015x; 1.1015x over previous
"""Bass/Trainium2 kernel for batched multi-head self-attention.

Module math (per batch b):
    q = vec @ Wq; k = vec @ Wk; v = vec @ Wv            (per head h, dim d=16)
    S = q k^T / sqrt(d);  P = softmax_j(S);  recv = P v
    out = recv @ Wo

Sharding: data-parallel over batch (8 batches -> 8 NeuronCores), weights
replicated. Each core runs an identical Bass program on its vec slice.

Pipeline structure (v2 — engine-balance rewrite):
  - Round r handles head pair (2r, 2r+1) in 64-partition strips of QT/KT.
  - 2-deep weave: round r's S'^T matmuls + exps are interleaved chunk-by-chunk
    with round r+1's max-pass matmuls + DVE row-max reduces, so ACT (exp),
    DVE (reduce) and PE (matmul) all stay busy.
  - m-dance is PSUM-free: the per-chunk -max columns land in m16 [128, 8]
    (fp16), a DVE 32x32 block-transpose gives mT, and one strided DMA
    flattens it into the fp16 aug row of QT. The aug row (ones x -max on the
    KT side) subtracts the row max inside the S'^T matmul, so the ACT exp
    with scale=1/4 needs no per-column bias.
  - PV: all four accumulation chains of a round (2 heads x 2 i-halves) live
    in ONE PSUM bank at column strips 0/32/64/96 (tile_position col tiling);
    evacuation is a single [128, 512] scalar copy per round.
  - A warm-up burst of fp32 matmuls at t=0 pushes the PE HAM activity window
    toward K=8/8 while the initial DMAs run.

Strip layout (64 rows/head): rows 0-15 q/k hi (fp16 RNE), row 16 aug
(ones for KT / -rowmax for QT), rows 32-47 q residual | k hi copy,
rows 48-63 q hi copy | k residual. The three live row-bands give
S = qhi*khi + qlo*khi + qhi*klo: ~22-bit precision at fp16 matmul rate.

Shapes (hardcoded): vec [8, 1024, 128]; Wq/Wk/Wv [128, 8, 16]; Wo [8, 16, 128].
"""

import sys

sys.path.insert(0, "/opt/trn_rl_repo")

from contextlib import ExitStack

import numpy as np

import concourse.bacc as bacc
import concourse.tile as tile
from concourse import mybir
from concourse.bass_utils import run_bass_kernel_spmd
from concourse.masks import make_identity

F32 = mybir.dt.float32
F16 = mybir.dt.float16
Exp = mybir.ActivationFunctionType.Exp

B, N, X, H, D = 8, 1024, 128, 8, 16
NCHUNK = N // 128          # 8 chunks of 128 along the token dim
SCALE = 0.25               # 1/sqrt(16)
NR = 4                     # rounds: 2 heads each at strips {0, 64}

_CACHED_NC = None


def build_nc():
    """Build the per-core Bass program (identical on all cores)."""
    nc = bacc.Bacc("TRN2")

    d_wq = [nc.dram_tensor(f"wq{r}", (X, 128), F32, kind="ExternalInput")
            for r in range(NR)]
    d_wk = [nc.dram_tensor(f"wk{r}", (X, 128), F32, kind="ExternalInput")
            for r in range(NR)]
    d_wv = nc.dram_tensor("wv", (X, 128), F32, kind="ExternalInput")
    d_wo = nc.dram_tensor("wo", (128, X), F32, kind="ExternalInput")
    d_vec = nc.dram_tensor("vec", (N, X), F32, kind="ExternalInput")
    d_e8 = nc.dram_tensor("e8c", (H, 128), F32, kind="ExternalInput")
    d_ones = nc.dram_tensor("ones", (1, N), F16, kind="ExternalInput")
    d_out = nc.dram_tensor("out", (N, X), F32, kind="ExternalOutput")

    with tile.TileContext(nc) as tc, ExitStack() as top:
        const = top.enter_context(tc.tile_pool(name="const", bufs=1))
        ident = const.tile([128, 128], F32)
        make_identity(nc, ident)

        w_sb = {}
        for name, dram in ([(f"wq{r}", d_wq[r]) for r in range(NR)]
                           + [(f"wk{r}", d_wk[r]) for r in range(NR)]
                           + [("wv", d_wv), ("wo", d_wo)]):
            t = const.tile([128, 128], F32, tag=f"w_{name}", name=f"w_{name}")
            eng = [nc.sync, nc.scalar, nc.gpsimd][len(w_sb) % 3]
            eng.dma_start(out=t[:], in_=dram[:, :])
            w_sb[name] = t

        vecT = const.tile([128, N], F32, tag="vecT")      # [x, n]
        QT = {r: const.tile([128, N], F16, tag=f"qt{r}", name=f"qt{r}")
              for r in range(NR)}
        KT = {r: const.tile([128, N], F16, tag=f"kt{r}", name=f"kt{r}")
              for r in range(NR)}
        # V layout: [128 j-in-chunk, jc, 17*h + d], col 17h+16 = ones.
        V_sb = const.tile([128, NCHUNK, 17 * H], F16, tag="vsb")
        recvT = const.tile([128, N], F32, tag="recvT")     # [(h d), i]
        recvN = const.tile([128, N], F32, tag="recvN")     # normalized
        # denominators packed over all 128 partitions: head h, token i ->
        # den128[16h + 8*half + (i-half*512)//64 ... flattened [1,512]->[8,64]
        den128 = const.tile([128, 64], F32, tag="den128")
        rden128 = const.tile([128, 64], F32, tag="rden128")
        rden = const.tile([H, N], F32, tag="rden")
        e8 = const.tile([H, 128], F32, tag="e8")           # expand matrix
        mha_sb = const.tile([128, NCHUNK, X], F32, tag="mha")

        pt_pool = top.enter_context(tc.tile_pool(name="pt", bufs=4))
        raw_pool = top.enter_context(tc.tile_pool(name="raw", bufs=2))
        mh_pool = top.enter_context(tc.tile_pool(name="mh", bufs=4))
        mt_pool = top.enter_context(tc.tile_pool(name="mt", bufs=2))

        psm = top.enter_context(tc.tile_pool(name="psm", bufs=3, space="PSUM"))
        psr = top.enter_context(tc.tile_pool(name="psr", bufs=2, space="PSUM"))

        # ---- PE warm-up burst: dense fp32 matmuls with no input deps so the
        # HAM SHORT window sees sustained activity while DMAs run. ----
        warm = psr.tile([128, 512], F32, tag="pv", name="warm")
        for i in range(8):
            nc.tensor.matmul(warm[:, 0:128], ident[:], ident[:],
                             start=True, stop=True)

        nc.sync.dma_start(out=e8[:], in_=d_e8[:, :])
        v_heads = V_sb[:].rearrange("p c (h s) -> p c h s", h=H)
        nc.vector.memset(v_heads[:, :, :, 16:17], 1.0)

        # ---- Phase 0: vecT via PE transposes; projections. ----
        with tc.tile_pool(name="stage", bufs=3) as stage:
            for c in range(NCHUNK):
                vt = stage.tile([128, 128], F32, tag="vstage")
                nc.sync.dma_start(out=vt[:], in_=d_vec[c * 128:(c + 1) * 128, :])
                pt_ = psr.tile([128, 512], F32, tag="pv", name=f"vtr{c}")
                nc.tensor.transpose(pt_[:, 0:128], vt[:], ident[:])
                nc.vector.tensor_copy(vecT[:, c * 128:(c + 1) * 128],
                                      pt_[:, 0:128])

            # QT/KT projections: psum = W.T @ vecT  -> [hd-pos, n]
            # fp16 hi/lo split: the plain copy rounds every strip row to
            # fp16 (hi); a tensor_tensor subtract then overwrites the
            # residual rows with fp16(p - hi).
            for rnd in range(NR):
                for wname, dst, is_q in ((f"wq{rnd}", QT[rnd], True),
                                         (f"wk{rnd}", KT[rnd], False)):
                    p = psm.tile([128, N], F32, tag="big", name=f"pj_{wname}")
                    for half in range(2):
                        sl = slice(half * 512, (half + 1) * 512)
                        nc.tensor.matmul(p[:, sl], w_sb[wname][:],
                                         vecT[:, sl], start=True, stop=True)
                    nc.scalar.copy(dst[:, :], p[:, :])
                    for t in range(2):
                        b = 64 * t
                        if is_q:
                            # residual rows 32-47 (16-row TT, 32-aligned)
                            rs = slice(b + 32, b + 48)
                            nc.vector.tensor_tensor(
                                dst[rs, :], p[rs, :], dst[rs, :],
                                op=mybir.AluOpType.subtract)
                        else:
                            # residual wanted at 48-63; only 32-aligned
                            # windows are legal, so TT 32-63 then restore
                            # hi over 32-47.
                            rs = slice(b + 32, b + 64)
                            nc.vector.tensor_tensor(
                                dst[rs, :], p[rs, :], dst[rs, :],
                                op=mybir.AluOpType.subtract)
                            nc.scalar.copy(dst[b + 32:b + 48, :],
                                           p[b + 32:b + 48, :])
            # ones rows of KT aug partitions
            for rnd in range(NR):
                for t in range(2):
                    nc.sync.dma_start(
                        out=KT[rnd][64 * t + 16:64 * t + 17, :],
                        in_=d_ones[:, :])

            # V projection: per chunk [j, hd] = vecT[:,chunk].T @ Wv
            for c in range(NCHUNK):
                pv = psr.tile([128, 512], F32, tag="pv", name=f"pjv{c}")
                nc.tensor.matmul(pv[:, 0:128], vecT[:, c * 128:(c + 1) * 128],
                                 w_sb["wv"][:], start=True, stop=True)
                dst = V_sb[:, c, :].rearrange("p (h s) -> p h s", h=H)
                src = pv[:, 0:128].rearrange("p (h d) -> p h d", h=H)
                nc.vector.tensor_copy(dst[:, :, 0:16], src[:])

        # ---- Main loop over head-pair rounds. ----
        def emit_form1(rnd, c, m_hs):
            """One i-chunk of the max pass for both heads of rnd.

            f1[i, j] = q_i . k_j (aug row of QT is still zero here, so it
            contributes nothing). DVE row-max (negated, fp16) -> m16 col c.
            """
            qt_, kt_ = QT[rnd], KT[rnd]
            f1s = {}
            for h in (2 * rnd, 2 * rnd + 1):
                sp = 64 * (h % 2)
                f1 = psm.tile([128, N], F32, tag="big", name=f"f1_{h}_{c}")
                f1s[h] = f1
                for half in range(2):
                    sl = slice(half * 512, (half + 1) * 512)
                    nc.tensor.matmul(
                        f1[:, sl],
                        qt_[sp:sp + 64, c * 128:(c + 1) * 128],
                        kt_[sp:sp + 64, sl], start=True, stop=True)
            for h in (2 * rnd, 2 * rnd + 1):
                nc.vector.tensor_reduce(
                    m_hs[h][:, c:c + 1], f1s[h][:, :],
                    axis=mybir.AxisListType.X,
                    op=mybir.AluOpType.max, negate=True)

        def new_mhs(rnd):
            # [128, 32] fp16; cols 0-7 hold the per-chunk -rowmax columns,
            # cols 8-31 are never read (needed only so the DVE 32x32 block
            # transpose has a full square to chew on).
            return {h: mh_pool.tile([128, 32], F16, tag="mh", name=f"mh{h}")
                    for h in (2 * rnd, 2 * rnd + 1)}

        # prologue: round 0 max-pass
        m_cur = new_mhs(0)
        for c in range(NCHUNK):
            emit_form1(0, c, m_cur)

        def emit_dance(rnd, m_hs):
            """-rowmax -> aug row of QT[rnd], PSUM-free.

            DVE 32x32 block transpose: mT[32b + c, q] = m16[32b + q, c].
            One DMA per 32-partition band b moves rows 32b..32b+8 of mT into
            the strided aug positions c*128 + 32b + q (multi-level partition
            source APs mis-lower in a single DMA, so four simple ones).
            """
            qt_ = QT[rnd]
            for h in (2 * rnd, 2 * rnd + 1):
                sp = 64 * (h % 2)
                mT = mt_pool.tile([128, 32], F16, tag="mt", name=f"mt{h}")
                nc.vector.transpose(mT[:], m_hs[h][:])
                aug = qt_[sp + 16:sp + 17, :].rearrange(
                    "p (c u) -> p c u", c=NCHUNK)
                for bb in range(4):
                    eng = nc.sync if bb % 2 == 0 else nc.scalar
                    eng.dma_start(out=aug[:, :, 32 * bb:32 * bb + 32],
                                  in_=mT[32 * bb:32 * bb + NCHUNK, :])

        emit_dance(0, m_cur)

        for rnd in range(NR):
            pair = (2 * rnd, 2 * rnd + 1)
            qt, kt = QT[rnd], KT[rnd]

            # S'^T + exp, strip-interleaved across the head pair,
            # with next round's max-pass chunks woven in.
            m_nxt = new_mhs(rnd + 1) if rnd + 1 < NR else None
            PTs = {h: pt_pool.tile([128, NCHUNK * N], F16, tag="pt",
                                   name=f"pt{h}")
                   for h in pair}
            for jc in range(NCHUNK):
                sts = {}
                for h in pair:
                    sp = 64 * (h % 2)
                    st = psm.tile([128, N], F32, tag="big",
                                  name=f"st_{h}_{jc}")
                    sts[h] = st
                    for half in range(2):
                        sl = slice(half * 512, (half + 1) * 512)
                        nc.tensor.matmul(
                            st[:, sl],
                            kt[sp:sp + 64, jc * 128:(jc + 1) * 128],
                            qt[sp:sp + 64, sl], start=True, stop=True)
                for h in pair:
                    nc.scalar.activation(
                        PTs[h][:, jc * N:jc * N + N], sts[h][:, :],
                        Exp, bias=0.0, scale=SCALE)
                if m_nxt is not None:
                    emit_form1(rnd + 1, jc, m_nxt)

            # next round's m-dance: its reduces completed during the weave;
            # the transposes + aug DMAs hide behind the PV burst below.
            if m_nxt is not None:
                emit_dance(rnd + 1, m_nxt)

            # PV burst: all 4 chains (2 heads x 2 halves) share ONE PSUM
            # bank at column strips 0/32/64/96. Chain-major order keeps each
            # accumulation group's has_written bits coherent.
            prv = psr.tile([128, 512], F32, tag="pv", name=f"prv{rnd}")
            for half in range(2):
                for hh, h in enumerate(pair):
                    cs = 32 * (2 * half + hh)
                    for jc in range(NCHUNK):
                        nc.tensor.matmul(
                            prv[cs:cs + 17, :],
                            V_sb[:, jc, 17 * h:17 * h + 17],
                            PTs[h][:, jc * N + half * 512:
                                    jc * N + (half + 1) * 512],
                            start=(jc == 0), stop=(jc == NCHUNK - 1),
                            tile_position=(0, cs))
            rawr = raw_pool.tile([128, 512], F32, tag="raw",
                                 name=f"raw{rnd}")
            nc.scalar.copy(rawr[:], prv[:])
            for half in range(2):
                for hh, h in enumerate(pair):
                    cs = 32 * (2 * half + hh)
                    hs = slice(half * 512, (half + 1) * 512)
                    nc.gpsimd.dma_start(out=recvT[16 * h:16 * h + 16, hs],
                                        in_=rawr[cs:cs + 16, :])
                    dp = 16 * h + 8 * half
                    nc.gpsimd.dma_start(out=den128[dp:dp + 8, :],
                                        in_=rawr[cs + 16:cs + 17, :])
            m_cur = m_nxt

        # ---- Tail: normalize + output projection. ----
        nc.vector.reciprocal(rden128[:], den128[:])
        for h in range(H):
            nc.sync.dma_start(out=rden[h:h + 1, :],
                              in_=rden128[16 * h:16 * h + 16, :])
        pe_ = psm.tile([128, N], F32, tag="big", name="expand")
        for half in range(2):
            sl = slice(half * 512, (half + 1) * 512)
            nc.tensor.matmul(pe_[:, sl], e8[:], rden[:, sl],
                             start=True, stop=True)
        nc.vector.tensor_mul(recvN[:], recvT[:], pe_[:, :])
        for c in range(NCHUNK):
            po = psr.tile([128, 512], F32, tag="pv", name=f"po{c}")
            nc.tensor.matmul(po[:, 0:128], recvN[:, c * 128:(c + 1) * 128],
                             w_sb["wo"][:], start=True, stop=True)
            nc.scalar.copy(mha_sb[:, c, :], po[:, 0:128])
            nc.sync.dma_start(out=d_out[c * 128:(c + 1) * 128, :],
                              in_=mha_sb[:, c, :])

    nc.finalize()
    return nc


def _permute_weights(Wq, Wk, Wv, Wo):
    """Numpy-side weight layout prep: strip-pack with +/- replica padding.

    Strip cols 17+s / 24+s (s<7) carry replicated head dims (q: same sign
    both; k: opposite signs) so their S contributions cancel exactly.
    Col 16 stays zero (aug slot), col 63 zero.
    """
    def strip_pack(W, heads, neg_second):
        out = np.zeros((X, 128), dtype=np.float32)
        for t, h in enumerate(heads):
            base = 64 * t
            # three copies of the head dims: hi term + residual source +
            # hi-again (rows 17-32 / 33-48 become hi/lo split on device)
            out[:, base:base + 16] = W[:, h, :]
            out[:, base + 32:base + 48] = W[:, h, :]
            out[:, base + 48:base + 64] = W[:, h, :]
            for s in range(7):
                out[:, base + 17 + s] = W[:, h, s]
                out[:, base + 24 + s] = (-1.0 if neg_second else 1.0) \
                    * W[:, h, s]
        return out

    e8c = np.zeros((H, 128), dtype=np.float32)
    for h in range(H):
        e8c[h, 16 * h:16 * h + 16] = 1.0
    d = dict(
        wv=np.ascontiguousarray(Wv.reshape(X, 128)),
        wo=np.ascontiguousarray(Wo.reshape(128, X)),
        e8c=e8c, ones=np.ones((1, N), dtype=np.float16),
    )
    for r in range(NR):
        d[f"wq{r}"] = strip_pack(Wq, [2 * r, 2 * r + 1], False)
        d[f"wk{r}"] = strip_pack(Wk, [2 * r, 2 * r + 1], True)
    return d


def kernel(Wq, Wk, Wv, Wo, vec, trace=False):
    global _CACHED_NC
    if _CACHED_NC is None:
        _CACHED_NC = build_nc()
    nc = _CACHED_NC

    w = _permute_weights(np.asarray(Wq, np.float32), np.asarray(Wk, np.float32),
                         np.asarray(Wv, np.float32), np.asarray(Wo, np.float32))
    vec = np.asarray(vec, np.float32)
    in_maps = [dict(w, vec=np.ascontiguousarray(vec[b])) for b in range(B)]
    res = run_bass_kernel_spmd(nc, in_maps, core_ids=list(range(B)),
                               trace=trace)
    out = np.stack([res.results[b]["out"] for b in range(B)])
    if trace:
        return out, res
    return out


# revision 8
# speedup vs baseline: 1.2635x; 1.1471x over previous
"""Bass/Trainium2 kernel for batched multi-head self-attention.

Module math (per batch b):
    q = vec @ Wq; k = vec @ Wk; v = vec @ Wv            (per head h, dim d=16)
    S = q k^T / sqrt(d);  P = softmax_j(S);  recv = P v
    out = recv @ Wo

Sharding: data-parallel over batch (8 batches -> 8 NeuronCores), weights
replicated. Each core runs an identical Bass program on its vec slice.

Pipeline structure (v4):
  - Round r handles head pair (2r, 2r+1) in 64-partition strips of QT/KT.
  - 3-stream weave: each chunk iteration of round r emits the S'^T matmuls +
    exps of round r, the max-pass matmuls + DVE row-max reduces of round r+1,
    and four accumulation matmuls of round r-1's PV chains, so ACT (exp),
    DVE (reduce) and PE (matmul) stay busy with no dead PV burst.
  - PSUM budget (8 banks): 2x st [128,1024] (4) + 3x f1 [128,512] (3) +
    1x PV accumulator [128,512] (1).
  - m-dance is PSUM-free: per-(chunk, j-half) negated maxes land in
    m16 [128, 0:16] fp16, a DVE min combines halves into cols 16:24, a DVE
    32x32 block-transpose gives mT, and 4 per-band DMAs flatten rows
    32b+16..+24 into the fp16 aug row of QT. The aug row (ones x -max on the
    KT side) subtracts the row max inside the S'^T matmul, so the ACT exp
    with scale=1/4 needs no per-column bias.
  - PV: all 4 chains of a round (2 heads x 2 i-halves) share ONE PSUM bank
    at column strips 0/32/64/96; evacuation is one [128,512] copy per round.
  - Q/K/V projection operands are float32r (1 cycle/col vs 4 for fp32
    at free-dim >= 256).
  - A warm-up burst of matmuls at t=0 pushes the PE HAM activity window
    toward K=8/8 while the initial DMAs run.

Strip layout (64 rows/head): rows 0-15 q/k hi (fp16 RNE), row 16 aug
(ones for KT / -rowmax for QT), rows 32-47 q residual | k hi copy,
rows 48-63 q hi copy | k residual. The three live row-bands give
S = qhi*khi + qlo*khi + qhi*klo: ~22-bit precision at fp16 matmul rate.

Shapes (hardcoded): vec [8, 1024, 128]; Wq/Wk/Wv [128, 8, 16]; Wo [8, 16, 128].
"""

import sys

sys.path.insert(0, "/opt/trn_rl_repo")

from contextlib import ExitStack

import numpy as np

import concourse.bacc as bacc
import concourse.tile as tile
from concourse import mybir
from concourse.bass_utils import run_bass_kernel_spmd
from concourse.masks import make_identity

F32 = mybir.dt.float32
F32R = mybir.dt.float32r
F16 = mybir.dt.float16
Exp = mybir.ActivationFunctionType.Exp

B, N, X, H, D = 8, 1024, 128, 8, 16
NCHUNK = N // 128          # 8 chunks of 128 along the token dim
SCALE = 0.25               # 1/sqrt(16)
NR = 4                     # rounds: 2 heads each at strips {0, 64}

_CACHED_NC = None


def build_nc():
    """Build the per-core Bass program (identical on all cores)."""
    nc = bacc.Bacc("TRN2")

    d_wq = [nc.dram_tensor(f"wq{r}", (X, 128), F32R, kind="ExternalInput")
            for r in range(NR)]
    d_wk = [nc.dram_tensor(f"wk{r}", (X, 128), F32R, kind="ExternalInput")
            for r in range(NR)]
    d_wv = nc.dram_tensor("wv", (X, 128), F32R, kind="ExternalInput")
    d_wo = nc.dram_tensor("wo", (128, X), F32, kind="ExternalInput")
    d_vec = nc.dram_tensor("vec", (N, X), F32, kind="ExternalInput")
    d_e8 = nc.dram_tensor("e8c", (H, 128), F32, kind="ExternalInput")
    d_ones = nc.dram_tensor("ones", (1, N), F16, kind="ExternalInput")
    d_out = nc.dram_tensor("out", (N, X), F32, kind="ExternalOutput")

    with tile.TileContext(nc) as tc, ExitStack() as top:
        const = top.enter_context(tc.tile_pool(name="const", bufs=1))
        ident = const.tile([128, 128], F32)
        make_identity(nc, ident)

        w_sb = {}
        for name, dram in ([(f"wq{r}", d_wq[r]) for r in range(NR)]
                           + [(f"wk{r}", d_wk[r]) for r in range(NR)]
                           + [("wv", d_wv), ("wo", d_wo)]):
            wdt = F32 if name == "wo" else F32R
            t = const.tile([128, 128], wdt, tag=f"w_{name}", name=f"w_{name}")
            eng = [nc.sync, nc.scalar, nc.gpsimd][len(w_sb) % 3]
            eng.dma_start(out=t[:], in_=dram[:, :])
            w_sb[name] = t

        vecT = const.tile([128, N], F32R, tag="vecT")      # [x, n]
        QT = {r: const.tile([128, N], F16, tag=f"qt{r}", name=f"qt{r}")
              for r in range(NR)}
        KT = {r: const.tile([128, N], F16, tag=f"kt{r}", name=f"kt{r}")
              for r in range(NR)}
        # V layout: [128 j-in-chunk, jc, 17*h + d], col 17h+16 = ones.
        V_sb = const.tile([128, NCHUNK, 17 * H], F16, tag="vsb")
        recvT = const.tile([128, N], F32, tag="recvT")     # [(h d), i]
        recvN = const.tile([128, N], F32, tag="recvN")     # normalized
        den128 = const.tile([128, 64], F32, tag="den128")
        rden128 = const.tile([128, 64], F32, tag="rden128")
        rden = const.tile([H, N], F32, tag="rden")
        e8 = const.tile([H, 128], F32, tag="e8")           # expand matrix
        mha_sb = const.tile([128, NCHUNK, X], F32, tag="mha")

        pt_pool = top.enter_context(tc.tile_pool(name="pt", bufs=4))
        raw_pool = top.enter_context(tc.tile_pool(name="raw", bufs=2))
        mh_pool = top.enter_context(tc.tile_pool(name="mh", bufs=4))
        mt_pool = top.enter_context(tc.tile_pool(name="mt", bufs=2))

        # PSUM: 2x[128,1024] st (4 banks) + 3x[128,512] f1 (3) + 1 PV (1).
        ps_st = top.enter_context(tc.tile_pool(name="ps_st", bufs=2,
                                               space="PSUM"))
        ps_f1 = top.enter_context(tc.tile_pool(name="ps_f1", bufs=3,
                                               space="PSUM"))
        ps_pv = top.enter_context(tc.tile_pool(name="ps_pv", bufs=1,
                                               space="PSUM"))

        # ---- PE warm-up burst: dense matmuls with no input deps so the
        # HAM SHORT window sees sustained activity while DMAs run. ----
        warm = ps_f1.tile([128, 512], F32, tag="f1", name="warm")
        for _ in range(12):
            nc.tensor.matmul(warm[:, 0:128], ident[:], ident[:],
                             start=True, stop=True)

        nc.sync.dma_start(out=e8[:], in_=d_e8[:, :])
        v_heads = V_sb[:].rearrange("p c (h s) -> p c h s", h=H)
        nc.vector.memset(v_heads[:, :, :, 16:17], 1.0)

        # ---- Phase 0: vecT via PE transposes; projections. ----
        with tc.tile_pool(name="stage", bufs=3) as stage:
            for c in range(NCHUNK):
                vt = stage.tile([128, 128], F32, tag="vstage")
                nc.sync.dma_start(out=vt[:], in_=d_vec[c * 128:(c + 1) * 128, :])
                pt_ = ps_f1.tile([128, 512], F32, tag="f1", name=f"vtr{c}")
                nc.tensor.transpose(pt_[:, 0:128], vt[:], ident[:])
                nc.vector.tensor_copy(vecT[:, c * 128:(c + 1) * 128],
                                      pt_[:, 0:128])

            # QT/KT projections: psum = W.T @ vecT  -> [hd-pos, n]
            # fp16 hi/lo split: the plain copy rounds every strip row to
            # fp16 (hi); a tensor_tensor subtract then overwrites the
            # residual rows with fp16(p - hi).
            for rnd in range(NR):
                for wname, dst, is_q in ((f"wq{rnd}", QT[rnd], True),
                                         (f"wk{rnd}", KT[rnd], False)):
                    p = ps_st.tile([128, N], F32, tag="st", name=f"pj_{wname}")
                    for half in range(2):
                        sl = slice(half * 512, (half + 1) * 512)
                        nc.tensor.matmul(p[:, sl], w_sb[wname][:],
                                         vecT[:, sl], start=True, stop=True)
                    nc.scalar.copy(dst[:, :], p[:, :])
                    for t in range(2):
                        b = 64 * t
                        if is_q:
                            # residual rows 32-47 (16-row TT, 32-aligned)
                            rs = slice(b + 32, b + 48)
                            nc.vector.tensor_tensor(
                                dst[rs, :], p[rs, :], dst[rs, :],
                                op=mybir.AluOpType.subtract)
                        else:
                            # residual wanted at 48-63; only 32-aligned
                            # windows are legal, so TT 32-63 then restore
                            # hi over 32-47.
                            rs = slice(b + 32, b + 64)
                            nc.vector.tensor_tensor(
                                dst[rs, :], p[rs, :], dst[rs, :],
                                op=mybir.AluOpType.subtract)
                            nc.scalar.copy(dst[b + 32:b + 48, :],
                                           p[b + 32:b + 48, :])
            # ones rows of KT aug partitions
            for rnd in range(NR):
                for t in range(2):
                    nc.sync.dma_start(
                        out=KT[rnd][64 * t + 16:64 * t + 17, :],
                        in_=d_ones[:, :])

            # V projection: per chunk [j, hd] = vecT[:,chunk].T @ Wv
            for c in range(NCHUNK):
                pv = ps_f1.tile([128, 512], F32, tag="f1", name=f"pjv{c}")
                nc.tensor.matmul(pv[:, 0:128], vecT[:, c * 128:(c + 1) * 128],
                                 w_sb["wv"][:], start=True, stop=True)
                dst = V_sb[:, c, :].rearrange("p (h s) -> p h s", h=H)
                src = pv[:, 0:128].rearrange("p (h d) -> p h d", h=H)
                nc.vector.tensor_copy(dst[:, :, 0:16], src[:])

        # ---- Main loop over head-pair rounds. ----
        def emit_form1(rnd, c, m_hs):
            """One i-chunk of the max pass for both heads of rnd.

            f1[i, j-half] = q_i . k_j (aug row of QT is still zero here).
            DVE row-max per half (negated, fp16) -> m16 col c + 8*jh.
            """
            qt_, kt_ = QT[rnd], KT[rnd]
            for jh in range(2):
                sl = slice(jh * 512, (jh + 1) * 512)
                f1s = {}
                for h in (2 * rnd, 2 * rnd + 1):
                    sp = 64 * (h % 2)
                    f1 = ps_f1.tile([128, 512], F32, tag="f1",
                                    name=f"f1_{h}_{c}_{jh}")
                    f1s[h] = f1
                    nc.tensor.matmul(
                        f1[:, :],
                        qt_[sp:sp + 64, c * 128:(c + 1) * 128],
                        kt_[sp:sp + 64, sl], start=True, stop=True)
                for h in (2 * rnd, 2 * rnd + 1):
                    nc.vector.tensor_reduce(
                        m_hs[h][:, c + 8 * jh:c + 8 * jh + 1], f1s[h][:, :],
                        axis=mybir.AxisListType.X,
                        op=mybir.AluOpType.max, negate=True)

        def new_mhs(rnd):
            # [128, 32] fp16; cols 0-15 hold per-(chunk, j-half) -rowmax,
            # cols 16-23 the min-combined -rowmax, cols 24-31 junk (the DVE
            # 32x32 block transpose needs a full square).
            return {h: mh_pool.tile([128, 32], F16, tag="mh", name=f"mh{h}")
                    for h in (2 * rnd, 2 * rnd + 1)}

        def emit_dance(rnd, m_hs):
            """-rowmax -> aug row of QT[rnd], PSUM-free.

            min of the negated half-maxes = negated full max. DVE 32x32
            block transpose: mT[32b + cc, q] = m16[32b + q, cc]; one DMA per
            32-partition band b moves rows 32b+16..+24 of mT into the
            strided aug positions c*128 + 32b + q.
            """
            qt_ = QT[rnd]
            for h in (2 * rnd, 2 * rnd + 1):
                sp = 64 * (h % 2)
                m16 = m_hs[h]
                nc.vector.tensor_tensor(m16[:, 16:24], m16[:, 0:8],
                                        m16[:, 8:16], op=mybir.AluOpType.min)
                mT = mt_pool.tile([128, 32], F16, tag="mt", name=f"mt{h}")
                nc.vector.transpose(mT[:], m16[:])
                aug = qt_[sp + 16:sp + 17, :].rearrange(
                    "p (c u) -> p c u", c=NCHUNK)
                for bb in range(4):
                    eng = nc.sync if bb % 2 == 0 else nc.scalar
                    eng.dma_start(out=aug[:, :, 32 * bb:32 * bb + 32],
                                  in_=mT[32 * bb + 16:32 * bb + 24, :])

        def emit_pv_step(rnd, PTs_r, prv, c):
            """One full PV chain (8 accumulating matmuls) of round rnd.

            Emitted on odd weave iterations only (chain s = c//2), keeping
            each chain contiguous in its bank and halving the PE tile-mode
            switches vs per-iteration interleaving.
            """
            if c % 2 == 0:
                return
            pair = (2 * rnd, 2 * rnd + 1)
            s = c // 2
            half, hh = s // 2, s % 2
            h = pair[hh]
            cs = 32 * s
            for jc in range(NCHUNK):
                nc.tensor.matmul(
                    prv[cs:cs + 17, :],
                    V_sb[:, jc, 17 * h:17 * h + 17],
                    PTs_r[h][:, jc * N + half * 512:
                             jc * N + (half + 1) * 512],
                    start=(jc == 0), stop=(jc == NCHUNK - 1),
                    tile_position=(0, cs))

        def emit_pv_evac(rnd, prv):
            pair = (2 * rnd, 2 * rnd + 1)
            rawr = raw_pool.tile([128, 512], F32, tag="raw", name=f"raw{rnd}")
            nc.scalar.copy(rawr[:], prv[:])
            for half in range(2):
                for hh, h in enumerate(pair):
                    cs = 32 * (2 * half + hh)
                    hs = slice(half * 512, (half + 1) * 512)
                    nc.gpsimd.dma_start(out=recvT[16 * h:16 * h + 16, hs],
                                        in_=rawr[cs:cs + 16, :])
                    dp = 16 * h + 8 * half
                    nc.gpsimd.dma_start(out=den128[dp:dp + 8, :],
                                        in_=rawr[cs + 16:cs + 17, :])

        # prologue: round 0 max-pass + dance
        m_cur = new_mhs(0)
        for c in range(NCHUNK):
            emit_form1(0, c, m_cur)
        emit_dance(0, m_cur)

        PTs_prev = None
        prv_prev = None
        for rnd in range(NR):
            pair = (2 * rnd, 2 * rnd + 1)
            qt, kt = QT[rnd], KT[rnd]

            m_nxt = new_mhs(rnd + 1) if rnd + 1 < NR else None
            PTs = {h: pt_pool.tile([128, NCHUNK * N], F16, tag="pt",
                                   name=f"pt{h}")
                   for h in pair}
            for jc in range(NCHUNK):
                sts = {}
                for h in pair:
                    sp = 64 * (h % 2)
                    st = ps_st.tile([128, N], F32, tag="st",
                                    name=f"st_{h}_{jc}")
                    sts[h] = st
                    for half in range(2):
                        sl = slice(half * 512, (half + 1) * 512)
                        nc.tensor.matmul(
                            st[:, sl],
                            kt[sp:sp + 64, jc * 128:(jc + 1) * 128],
                            qt[sp:sp + 64, sl], start=True, stop=True)
                for h in pair:
                    nc.scalar.activation(
                        PTs[h][:, jc * N:jc * N + N], sts[h][:, :],
                        Exp, bias=0.0, scale=SCALE)
                if m_nxt is not None:
                    emit_form1(rnd + 1, jc, m_nxt)
                if PTs_prev is not None:
                    emit_pv_step(rnd - 1, PTs_prev, prv_prev, jc)
            if PTs_prev is not None:
                emit_pv_evac(rnd - 1, prv_prev)
            if m_nxt is not None:
                emit_dance(rnd + 1, m_nxt)

            prv_prev = ps_pv.tile([128, 512], F32, tag="pv", name=f"prv{rnd}")
            PTs_prev = PTs
            m_cur = m_nxt

        # final round's PV as a tail burst
        for c in range(1, NCHUNK, 2):
            emit_pv_step(NR - 1, PTs_prev, prv_prev, c)
        emit_pv_evac(NR - 1, prv_prev)

        # ---- Tail: normalize + output projection. ----
        nc.vector.reciprocal(rden128[:], den128[:])
        for h in range(H):
            nc.sync.dma_start(out=rden[h:h + 1, :],
                              in_=rden128[16 * h:16 * h + 16, :])
        pe_ = ps_st.tile([128, N], F32, tag="st", name="expand")
        for half in range(2):
            sl = slice(half * 512, (half + 1) * 512)
            nc.tensor.matmul(pe_[:, sl], e8[:], rden[:, sl],
                             start=True, stop=True)
        nc.vector.tensor_mul(recvN[:], recvT[:], pe_[:, :])
        for c in range(NCHUNK):
            po = ps_f1.tile([128, 512], F32, tag="f1", name=f"po{c}")
            nc.tensor.matmul(po[:, 0:128], recvN[:, c * 128:(c + 1) * 128],
                             w_sb["wo"][:], start=True, stop=True)
            nc.scalar.copy(mha_sb[:, c, :], po[:, 0:128])
            nc.sync.dma_start(out=d_out[c * 128:(c + 1) * 128, :],
                              in_=mha_sb[:, c, :])

    nc.finalize()
    return nc


def _permute_weights(Wq, Wk, Wv, Wo):
    """Numpy-side weight layout prep: strip-pack with +/- replica padding.

    Strip cols 17+s / 24+s (s<7) carry replicated head dims (q: same sign
    both; k: opposite signs) so their S contributions cancel exactly.
    Col 16 stays zero (aug slot).
    """
    def strip_pack(W, heads, neg_second):
        out = np.zeros((X, 128), dtype=np.float32)
        for t, h in enumerate(heads):
            base = 64 * t
            out[:, base:base + 16] = W[:, h, :]
            out[:, base + 32:base + 48] = W[:, h, :]
            out[:, base + 48:base + 64] = W[:, h, :]
            for s in range(7):
                out[:, base + 17 + s] = W[:, h, s]
                out[:, base + 24 + s] = (-1.0 if neg_second else 1.0) \
                    * W[:, h, s]
        return out

    e8c = np.zeros((H, 128), dtype=np.float32)
    for h in range(H):
        e8c[h, 16 * h:16 * h + 16] = 1.0
    d = dict(
        wv=np.ascontiguousarray(Wv.reshape(X, 128)),
        wo=np.ascontiguousarray(Wo.reshape(128, X)),
        e8c=e8c, ones=np.ones((1, N), dtype=np.float16),
    )
    for r in range(NR):
        d[f"wq{r}"] = strip_pack(Wq, [2 * r, 2 * r + 1], False)
        d[f"wk{r}"] = strip_pack(Wk, [2 * r, 2 * r + 1], True)
    return d


def kernel(Wq, Wk, Wv, Wo, vec, trace=False):
    global _CACHED_NC
    if _CACHED_NC is None:
        _CACHED_NC = build_nc()
    nc = _CACHED_NC

    w = _permute_weights(np.asarray(Wq, np.float32), np.asarray(Wk, np.float32),
                         np.asarray(Wv, np.float32), np.asarray(Wo, np.float32))
    vec = np.asarray(vec, np.float32)
    in_maps = [dict(w, vec=np.ascontiguousarray(vec[b])) for b in range(B)]
    res = run_bass_kernel_spmd(nc, in_maps, core_ids=list(range(B)),
                               trace=trace)
    out = np.stack([res.results[b]["out"] for b in range(B)])
    if trace:
        return out, res
    return out
